# revision 1
# baseline (speedup 1.0000x reference)
"""Contrastive loss kernel for Trainium2, SPMD across 8 NeuronCores.

Problem: embeddings [8192, 256] f32 -> L2-normalize rows, sim = e @ e.T,
loss = sum(relu(sim - 0.5) over strict upper triangle) / C(8192,2).

Distribution: a balanced round-robin "tournament" over the 8 row-slabs of
1024 rows. Core c receives a gathered column matrix of 5120 rows:
  cols[0:1024]    = its own slab S_c          (strict-upper pairs via masks)
  cols[1024:4096] = S_{c+1}, S_{c+2}, S_{c+3} (full cross blocks)
  cols[4096:5120] = half-coverage split of the distance-4 pair {c, c+4}
Every unordered pair (i < j) is counted exactly once across the 8 cores and
each core computes exactly 68 [128, 512] similarity tiles — perfectly
balanced and a single SPMD program (all per-core divergence lives in the
host-side gather, never in instruction addresses).

Per core: normalize rows in fp32 (square+rowsum via scalar_tensor_tensor on
VectorE, batched sqrt on ScalarE, reciprocal on VectorE, scale on GpSimd),
transpose via TensorE into 20 eT segment tiles [128, 512] bf16 (segmented so
phase-2 matmuls depend only on the columns they read and overlap the
normalize/transpose tail), then for each of 8 row-blocks matmul the needed
column tiles in 2-bank PSUM chunks, mask diagonal-straddling tiles (mask
multiply before the relu: relu(0 - 0.5) = 0, exact exclusion), and fuse
relu+row-sum alternately on ScalarE (activation accum_out) and VectorE
(tensor_scalar subtract/max accum_out) to split the reduction load.
Output: [128, 1] per-core partials; host adds 1024 numbers.

bf16 is safe: the margin slack (max off-diag sim ~0.435 vs 0.5) is two
orders of magnitude above bf16 dot-product error, so every masked relu term
is exactly 0.0 both here and in the fp32 reference.
"""

import numpy as np

import concourse.bass as bass
import concourse.bacc as bacc
import concourse.mybir as mybir
from concourse import masks
from concourse.tile import TileContext
from concourse.bass_utils import run_bass_kernel_spmd

N = 8192
D = 256
NCORES = 8
SLAB = N // NCORES  # 1024
LOCAL_COLS = SLAB + 3 * SLAB + SLAB  # 5120
MT = SLAB // 128  # 8 row blocks per core
CT = LOCAL_COLS // 128  # 40 column tiles to normalize+transpose
NG = CT // 4  # 10 load/transpose groups of 4 tiles = 10 eT segments
MARGIN = 0.5
CHUNK = 2  # n-tiles (512 cols) per PSUM chunk; 2 banks

_CACHE = {}


def _n_tiles(k):
    """Column-tile (width 512) indices for row-block k (local rows k*128..)."""
    # own slab: tiles k//4 .. 1 (k//4 is the diagonal-straddling tile);
    # cross slabs: tiles 2..7; distance-4 half block: tile 8 (rows < 512)
    # or tile 9 (rows >= 512).
    return list(range(k // 4, 8)) + [8 if k < 4 else 9]


def _chunks(k):
    nl = _n_tiles(k)
    return [nl[i : i + CHUNK] for i in range(0, len(nl), CHUNK)]


def _build_program():
    # Bacc (not raw Bass): its compile pipeline splits semaphore waits
    # (move_matmul_waits_to_ldweights / generate_event_semaphores) to satisfy
    # the 1-wait-per-instruction hardware constraint.
    nc = bacc.Bacc()
    emb = nc.declare_dram_parameter(
        "emb", [LOCAL_COLS, D], mybir.dt.float32, isOutput=False
    )
    out = nc.declare_dram_parameter("out", [128, 1], mybir.dt.float32, isOutput=True)

    n_chunks = sum(len(_chunks(k)) for k in range(MT))  # 36

    with TileContext(nc) as tc:
        with (
            tc.tile_pool(name="singles", bufs=1) as singles,
            tc.tile_pool(name="xin", bufs=4) as xin,
            tc.tile_pool(name="nrms", bufs=4) as nrms,
            tc.tile_pool(name="scr", bufs=2) as scr,
            tc.tile_pool(name="ract", bufs=3) as ract,
            tc.tile_pool(name="tpsum", bufs=2, space="PSUM") as tpsum,
            tc.tile_pool(name="mpsum", bufs=3, space="PSUM") as mpsum,
        ):
            ident = singles.tile([128, 128], mybir.dt.float32, tag="ident")
            masks.make_identity(nc, ident[:])

            # strict-upper-triangle masks for the 4 diagonal offsets
            dmask = []
            for pat in range(4):
                mk = singles.tile(
                    [128, 512], mybir.dt.float32, name=f"mask{pat}", tag=f"mask{pat}"
                )
                nc.gpsimd.memset(mk[:], 0.0)
                # mask[p, f] = 1.0 iff f > p + 128*pat (strict upper), else 0
                nc.gpsimd.affine_select(
                    out=mk[:],
                    in_=mk[:],
                    compare_op=mybir.AluOpType.is_ge,
                    fill=1.0,
                    base=128 * pat,
                    channel_multiplier=1,
                    pattern=[[-1, 512]],
                )
                dmask.append(mk)

            neg_margin = singles.tile([128, 1], mybir.dt.float32, tag="neg_margin")
            nc.gpsimd.memset(neg_margin[:], -MARGIN)
            zeros = singles.tile([128, CHUNK * 512], mybir.dt.float32, tag="zeros")
            nc.gpsimd.memset(zeros[:], 0.0)

            # eT segments: eTs[h][g] holds features [h*128,(h+1)*128) x
            # normalized columns [g*512,(g+1)*512), bf16
            eTs = [
                [
                    singles.tile(
                        [128, 512],
                        mybir.dt.bfloat16,
                        name=f"eT{h}_{g}",
                        tag=f"eT{h}_{g}",
                    )
                    for g in range(NG)
                ]
                for h in range(2)
            ]
            # DVE also carries squares/copies/masks (~32us busy): give it only
            # every 7th relu chunk, ACT the rest -> ACT/DVE/PE all land ~38us
            is_dve = lambda c: c % 7 == 3
            n_dve = sum(1 for c in range(n_chunks) if is_dve(c))
            n_act = n_chunks - n_dve
            acc_act = singles.tile([128, n_act], mybir.dt.float32, tag="acc_act")
            acc_dve = singles.tile([128, n_dve], mybir.dt.float32, tag="acc_dve")

            # [g, p, q, d] view: group g holds row tiles 4g..4g+3
            emb_g = emb.rearrange("(g q p) d -> g p q d", p=128, q=4)

            # ---- Phase 1: load, normalize rows (fp32), transpose to eT (bf16)
            for g in range(NG):
                xb = xin.tile([128, 4, D], mybir.dt.float32, tag="xb")
                nc.sync.dma_start(xb[:], emb_g[g])
                ssq = nrms.tile([128, 4], mybir.dt.float32, tag="ssq")
                for qi in range(4):
                    sqt = scr.tile([128, D], mybir.dt.float32, tag="sqt")
                    # (x * 1.0) * x with fused row-sum: one DVE pass for the
                    # squared row norms (tensor_tensor_reduce faults the exec
                    # unit on HW; this lowering works)
                    nc.vector.scalar_tensor_tensor(
                        out=sqt[:],
                        in0=xb[:, qi, :],
                        scalar=1.0,
                        in1=xb[:, qi, :],
                        op0=mybir.AluOpType.mult,
                        op1=mybir.AluOpType.mult,
                        accum_out=ssq[:, qi : qi + 1],
                    )
                nrm = nrms.tile([128, 4], mybir.dt.float32, tag="nrm")
                nc.scalar.activation(
                    nrm[:], ssq[:], mybir.ActivationFunctionType.Sqrt
                )
                # torch F.normalize eps clamp
                nc.vector.tensor_scalar_max(nrm[:], nrm[:], 1e-12)
                rinv = nrms.tile([128, 4], mybir.dt.float32, tag="rinv")
                nc.vector.reciprocal(rinv[:], nrm[:])
                et = xin.tile([128, 4 * D], mybir.dt.float32, tag="et")
                for qi in range(4):
                    nc.gpsimd.tensor_scalar_mul(
                        et[:, qi * D : (qi + 1) * D],
                        xb[:, qi, :],
                        rinv[:, qi : qi + 1],
                    )
                for h in range(2):
                    pst = tpsum.tile([128, 512], mybir.dt.float32, tag="pst")
                    for qi in range(4):
                        nc.tensor.transpose(
                            pst[:, qi * 128 : (qi + 1) * 128],
                            et[:, qi * D + h * 128 : qi * D + h * 128 + 128],
                            ident[:],
                        )
                    nc.vector.tensor_copy(eTs[h][g][:], pst[:])

            # ---- Phase 2: sim tiles in 2-bank PSUM chunks, relu+row-sum
            # alternating between ScalarE and VectorE
            col = 0
            col_a = 0
            col_d = 0
            for k in range(MT):
                lg, lo = k // 4, (k % 4) * 128
                for chunk in _chunks(k):
                    used = len(chunk) * 512
                    pg = mpsum.tile([128, CHUNK * 512], mybir.dt.float32, tag="pg")
                    for ci, n in enumerate(chunk):
                        psl = slice(ci * 512, (ci + 1) * 512)
                        for h in range(2):
                            nc.tensor.matmul(
                                pg[:, psl],
                                eTs[h][lg][:, lo : lo + 128],
                                eTs[h][n][:],
                                start=(h == 0),
                                stop=(h == 1),
                            )
                        if n == k // 4:
                            # diagonal-straddling tile: zero i >= j before the
                            # relu (relu(0 - 0.5) = 0 -> exact exclusion)
                            nc.vector.tensor_mul(
                                pg[:, psl], pg[:, psl], dmask[k % 4][:]
                            )
                    rs = ract.tile([128, CHUNK * 512], mybir.dt.float32, tag="rs")
                    if not is_dve(col):
                        nc.scalar.activation(
                            rs[:, :used],
                            pg[:, :used],
                            mybir.ActivationFunctionType.Relu,
                            bias=neg_margin[:],
                            accum_out=acc_act[:, col_a : col_a + 1],
                        )
                        col_a += 1
                    else:
                        # (pg - 0.5) max 0 with accum_out = SUM(out);
                        # tensor_scalar's accum_out reduces with op1 (max),
                        # so it cannot produce the row-sum — stt can.
                        nc.vector.scalar_tensor_tensor(
                            out=rs[:, :used],
                            in0=pg[:, :used],
                            scalar=MARGIN,
                            in1=zeros[:, :used],
                            op0=mybir.AluOpType.subtract,
                            op1=mybir.AluOpType.max,
                            accum_out=acc_dve[:, col_d : col_d + 1],
                        )
                        col_d += 1
                    col += 1

            acc2 = singles.tile([128, 2], mybir.dt.float32, tag="acc2")
            nc.vector.reduce_sum(
                acc2[:, 0:1], acc_act[:, 0:col_a], axis=mybir.AxisListType.X
            )
            nc.vector.reduce_sum(
                acc2[:, 1:2], acc_dve[:, 0:col_d], axis=mybir.AxisListType.X
            )
            accsum = singles.tile([128, 1], mybir.dt.float32, tag="accsum")
            nc.vector.reduce_sum(accsum[:], acc2[:], axis=mybir.AxisListType.X)
            nc.sync.dma_start(out[:], accsum[:])

    nc.finalize()
    return nc


def _gather_cols(x, c):
    """Column matrix [5120, 256] for core c (see module docstring)."""
    s = lambda i: x[(i % NCORES) * SLAB : (i % NCORES) * SLAB + SLAB]
    partner = s(c + 4)
    if c < 4:
        tail = partner
    else:
        tail = np.concatenate([partner[512:], partner[:512]], axis=0)
    return np.ascontiguousarray(
        np.concatenate([s(c), s(c + 1), s(c + 2), s(c + 3), tail], axis=0)
    )


def kernel(embeddings):
    x = np.ascontiguousarray(np.asarray(embeddings), dtype=np.float32)
    assert x.shape == (N, D)

    if "nc" not in _CACHE:
        _CACHE["nc"] = _build_program()
    nc = _CACHE["nc"]

    in_maps = [{"emb": _gather_cols(x, c)} for c in range(NCORES)]
    res = run_bass_kernel_spmd(nc, in_maps, core_ids=list(range(NCORES)))

    total = 0.0
    for c in range(NCORES):
        total += float(np.asarray(res.results[c]["out"], dtype=np.float64).sum())
    count = N * (N - 1) // 2
    return np.float32(total / count)

